# revision 32
# baseline (speedup 1.0000x reference)
"""EuclideanFastAttention Trainium2 kernel.

Full inputs -> shard graphs across 8 NeuronCores (1 graph/core) -> per-core
Bass/Tile kernel (Euclidean RoPE + linear attention over Lebedev quadrature)
-> gather full output.

Self-contained: hardcodes the problem geometry (N=2048, B=8, P=1, S=4, F=64,
G=14, J=32) but derives everything it can from the input arrays at runtime.
"""
import sys

sys.path.insert(0, "/opt/trn_rl_repo")

import numpy as np

import concourse.bacc as bacc
import concourse.bass as bass
import concourse.mybir as mybir
import concourse.tile as tile
from concourse import masks
from concourse.bass_utils import run_bass_kernel_spmd

F32 = mybir.dt.float32
F32R = mybir.dt.float32r
ACTF = mybir.ActivationFunctionType
ALU = mybir.AluOpType

PI = float(np.pi)
TWO_PI = float(2.0 * np.pi)
INV_2PI = float(1.0 / (2.0 * np.pi))
MAGIC = float(1.5 * 2.0**23)  # fp32 round-to-nearest-int magic constant

N_CORES = 8


def _bcast2(ap_2d, reps):
    """Read-broadcast a [P, M] AP to [P, reps, M] via a step-0 middle dim."""
    a = ap_2d
    return bass.AP(
        tensor=a.tensor,
        offset=a.offset,
        ap=[list(a.ap[0]), [0, reps], list(a.ap[1])],
    )


def _build_program(nk, terms, G, J, mm_dt=F32R):
    """Build the SPMD per-core program.

    nk:    number of 128-node chunks per core (M_cap = 128*nk)
    terms: list of tuples; (i, j) = antipodal pair (u_j = -u_i, w_j = w_i),
           (s,) = single grid direction.
    """
    D = 256
    M = 128 * nk
    # aux blob column layout
    c_mask = 0
    c_post = nk
    c_ut = c_post + M
    c_frq = c_ut + G
    c_w = c_frq + J
    W = c_w + G

    nc = bacc.Bacc()
    X = nc.declare_dram_parameter("x", [128, nk * D], F32, isOutput=False)
    AUX = nc.declare_dram_parameter("aux", [128, W], F32, isOutput=False)
    OUT = nc.declare_dram_parameter("out", [128, nk * D], F32, isOutput=True)

    with tile.TileContext(nc) as tc:
        with (
            tc.tile_pool(name="const", bufs=1) as cp,
            tc.tile_pool(name="work", bufs=4) as wp,
            tc.tile_pool(name="trps", bufs=2, space="PSUM") as trps,
            tc.tile_pool(name="kvps", bufs=(3 if nk <= 2 else 2), space="PSUM") as kvps,
            tc.tile_pool(name="outps", bufs=1, space="PSUM") as outps,
        ):
            # ---------------- setup ----------------
            x_sb = cp.tile([128, nk * D], F32)
            aux_sb = cp.tile([128, W], F32)
            # rows 0:3 of cols nk.. carry posT/uT/freq/grid_w -> the whole
            # angle-pipeline prefix; land them first in a tiny DMA
            nc.sync.dma_start(out=aux_sb[0:3, c_post:W], in_=AUX[0:3, c_post:W])
            nc.sync.dma_start(out=aux_sb[:, 0:nk], in_=AUX[:, 0:nk])
            nc.sync.dma_start(out=x_sb, in_=X[:, :])

            ident = cp.tile([128, 128], F32)
            masks.make_identity(nc, ident[:])
            ident_r = cp.tile([128, 128], F32R)
            nc.vector.tensor_copy(ident_r, ident)

            zero_col = cp.tile([128, 1], F32)
            nc.vector.memset(zero_col, 0.0)
            halfpi_col = cp.tile([128, 1], F32)
            nc.vector.memset(halfpi_col, PI / 2)

            # dotposT[g, n] = sum_c uT[c, g] * posT[c, n]
            dp_ps = trps.tile([G, M], F32, tag="tp")
            nc.tensor.matmul(
                dp_ps,
                aux_sb[0:3, c_ut : c_ut + G],
                aux_sb[0:3, c_post : c_post + M],
                start=True,
                stop=True,
            )
            dp_sb = cp.tile([G, M], F32)
            nc.vector.tensor_copy(dp_sb, dp_ps)
            # collapse [G, M] partitions into one row (partition_broadcast
            # can only read partition 0)
            dp_row = cp.tile([1, G * M], F32)
            nc.sync.dma_start(out=dp_row[0:1, :], in_=dp_sb[:, :])

            # freq_col[p] = freq[(p % 64)//2] / (2*pi), via row build + matmul
            frow = cp.tile([1, 128], F32)
            fv = frow[:].rearrange("p (a j two) -> p a j two", a=2, two=2)
            for a in range(2):
                for t in range(2):
                    nc.vector.tensor_copy(
                        fv[0:1, a, :, t], aux_sb[0:1, c_frq : c_frq + J]
                    )
            ones1 = cp.tile([1, 1], F32)
            nc.vector.memset(ones1, 1.0)
            fc_ps = trps.tile([128, 1], F32, tag="tp")
            nc.tensor.matmul(fc_ps, frow, ones1, start=True, stop=True)
            f2pi_col = cp.tile([128, 1], F32)
            nc.vector.tensor_scalar_mul(f2pi_col, fc_ps, INV_2PI)  # psum read: DVE

            # grid weights broadcast; per-term weight columns
            w_bc = cp.tile([128, G], F32)
            nc.gpsimd.partition_broadcast(w_bc, aux_sb[0:1, c_w : c_w + G])
            w_term = cp.tile([128, len(terms)], F32)
            for t, term in enumerate(terms):
                if len(term) == 2:
                    i, j = term
                    nc.gpsimd.tensor_add(
                        w_term[:, t : t + 1], w_bc[:, i : i + 1], w_bc[:, j : j + 1]
                    )
                else:
                    (s,) = term
                    nc.gpsimd.tensor_copy(w_term[:, t : t + 1], w_bc[:, s : s + 1])

            # xswap natural: xsw[:, 2q] = -x[:, 2q+1], xsw[:, 2q+1] = x[:, 2q]
            xsw = cp.tile([128, nk * D], F32)
            xv = x_sb[:].rearrange("p (q two) -> p q two", two=2)
            sv = xsw[:].rearrange("p (q two) -> p q two", two=2)
            nc.scalar.activation(sv[:, :, 0:1], xv[:, :, 1:2], ACTF.Copy, scale=-1.0)
            nc.scalar.activation(sv[:, :, 1:2], xv[:, :, 0:1], ACTF.Copy)

            # transposed copies into one tile: xTc[:, 0:2M] = xT (x transposed,
            # [dchunk partition, n free]); xTc[:, 2M:4M] = xswT (xsw transposed)
            xTc = cp.tile([128, 4 * M], F32)
            for si, src in enumerate((x_sb, xsw)):
                tp = trps.tile([128, 512], F32, tag="tp", name="tp_setup")
                for c in range(nk):
                    for dc in range(2):
                        nc.tensor.transpose(
                            tp[:, (c * 2 + dc) * 128 : (c * 2 + dc) * 128 + 128],
                            src[:, c * D + dc * 128 : c * D + dc * 128 + 128],
                            ident,
                        )
                # single permuting copy: (c dc l) -> (dc c l)
                nc.scalar.activation(
                    xTc[:, si * 2 * M : (si + 1) * 2 * M].rearrange(
                        "p (dc c l) -> p dc c l", dc=2, c=nk
                    ),
                    tp[:, : nk * 256].rearrange("p (c dc l) -> p dc c l", c=nk, dc=2),
                    ACTF.Copy,
                )

            # v = x * mask, rounded to matmul dtype
            vmask = cp.tile([128, nk * D], mm_dt)
            for c in range(nk):
                nc.scalar.activation(
                    vmask[:, c * D : (c + 1) * D],
                    x_sb[:, c * D : (c + 1) * D],
                    ACTF.Copy,
                    scale=aux_sb[:, c_mask + c : c_mask + c + 1],
                )

            # persistent output accumulators packed into one PSUM bank;
            # per-chunk groups interleave in one zero region, so the group
            # check is skipped on the out matmuls (has_written is per element)
            out_bank = outps.tile([128, nk * D], F32, tag="out_bank",
                                  name="out_bank")
            outp = [out_bank[:, c * D : (c + 1) * D] for c in range(nk)]
            # start=True zeroes the whole PSUM zero region (one 2KB bank =
            # two 256-col chunks), so exactly the first matmul into each bank
            # carries start=True and the last one carries stop=True
            n_banks = (nk + 1) // 2
            bank_of = [c // 2 for c in range(nk)]
            mm_count = [0] * n_banks
            mm_total = [0] * n_banks
            for term in terms:
                for c in range(nk):
                    mm_total[bank_of[c]] += 4 if len(term) == 2 else 2

            # ---- software-pipelined main loop: B(t) | C(t-1) | D(t-2) ----
            # B: angles + fused sin/cos   (Pool/DVE + ACT)
            # C: fused rotation t12       (DVE)
            # D: transposes + kv + out    (PE + DVE/ACT psum copies)
            sc_t = {}
            t12_t = {}
            parts_t = {}

            def emit_B(t, term):
                i = term[0]
                dpb = wp.tile([128, M], F32, tag="dpb", name="dpb")
                if i == 0:
                    # row 0 lives on partition 0 of dp_sb already; skip the
                    # dp_row collapse dependency for the first term
                    nc.gpsimd.partition_broadcast(dpb, dp_sb[0:1, :])
                else:
                    nc.gpsimd.partition_broadcast(
                        dpb, dp_row[0:1, i * M : (i + 1) * M]
                    )
                eng = nc.gpsimd if t % 2 == 0 else nc.vector
                eng2 = eng
                w_t = wp.tile([128, M], F32, tag="w_t", name="w_t")
                eng.tensor_scalar_mul(w_t, dpb, f2pi_col[:, 0:1])
                k_sc = wp.tile([128, 2 * M], F32, tag="k_sc", name="k_sc")
                eng2.tensor_scalar(
                    k_sc[:, 0:M], w_t, MAGIC, MAGIC, ALU.add, ALU.subtract
                )
                t_c = wp.tile([128, M], F32, tag="t_c", name="t_c")
                eng.tensor_scalar(t_c, w_t, 0.25, MAGIC, ALU.add, ALU.add)
                eng.tensor_scalar(
                    k_sc[:, M : 2 * M], t_c, MAGIC, 0.25, ALU.subtract,
                    ALU.subtract,
                )
                d_sc = wp.tile([128, 2 * M], F32, tag="d_sc", name="d_sc")
                (nc.gpsimd if t % 2 == 0 else nc.vector).tensor_sub(
                    d_sc[:].rearrange("p (two m) -> p two m", two=2),
                    _bcast2(w_t[:], 2),
                    k_sc[:].rearrange("p (two m) -> p two m", two=2),
                )
                sc = wp.tile([128, 2 * M], F32, tag="sc", name="sc")
                nc.scalar.activation(
                    sc, d_sc, ACTF.Sin, bias=zero_col[:, 0:1], scale=TWO_PI
                )
                sc_t[t] = sc

            def emit_C(t, term):
                t12 = wp.tile([128, 4 * M], mm_dt, tag="t12", name="t12")
                sc_ap = sc_t.pop(t)[:]
                if t == 0:
                    # split so part0's transposes can start one op earlier
                    nc.vector.tensor_mul(
                        t12[:, 0 : 2 * M].rearrange("p (dc m) -> p dc m", dc=2),
                        xTc[:, 0 : 2 * M].rearrange("p (dc m) -> p dc m", dc=2),
                        _bcast2(sc_ap[:, M : 2 * M], 2),
                    )
                    nc.vector.tensor_mul(
                        t12[:, 2 * M : 4 * M].rearrange("p (dc m) -> p dc m", dc=2),
                        xTc[:, 2 * M : 4 * M].rearrange("p (dc m) -> p dc m", dc=2),
                        _bcast2(sc_ap[:, 0:M], 2),
                    )
                else:
                    sc_in = bass.AP(
                        tensor=sc_ap.tensor,
                        offset=sc_ap.offset + M,
                        ap=[list(sc_ap.ap[0]), [-M, 2], [0, 2], [1, M]],
                    )
                    nc.vector.tensor_mul(
                        t12[:].rearrange("p (h dc m) -> p h dc m", h=2, dc=2),
                        xTc[:].rearrange("p (h dc m) -> p h dc m", h=2, dc=2),
                        sc_in,
                    )
                t12_t[t] = t12
                if len(term) == 2:
                    parts_t[t] = [t12[:, 0 : 2 * M], t12[:, 2 * M : 4 * M]]
                else:
                    rT = wp.tile([128, 2 * M], mm_dt, tag="rT", name="rT")
                    nc.vector.tensor_add(
                        rT, t12[:, 0 : 2 * M], t12[:, 2 * M : 4 * M]
                    )
                    parts_t[t] = [rT]

            def emit_D(t, term):
                parts = parts_t.pop(t)
                t12_t.pop(t, None)
                np_ = len(parts)
                tp = trps.tile([128, np_ * 512], mm_dt, tag="tp", name="tp_main")
                for pi_, pt in enumerate(parts):
                    for c in range(nk):
                        for dc in range(2):
                            col = pi_ * 512 + c * D + dc * 128
                            nc.tensor.matmul(
                                tp[:, col : col + 128],
                                pt[:, dc * M + c * 128 : dc * M + c * 128 + 128],
                                ident_r,
                                is_transpose=True,
                            )
                pnats = []
                for pi_ in range(np_):
                    pnat = wp.tile([128, nk * D], mm_dt, tag=f"nat{pi_}",
                                   name="pnat")
                    if pi_ == 0:
                        nc.scalar.activation(
                            pnat, tp[:, pi_ * 512 : pi_ * 512 + nk * D], ACTF.Copy
                        )
                    else:
                        nc.vector.tensor_copy(
                            pnat, tp[:, pi_ * 512 : pi_ * 512 + nk * D]
                        )
                    pnats.append(pnat)

                kvs = []
                for pi_ in range(np_):
                    kv_ps = kvps.tile([128, 512], F32, tag="kv_ps", name="kv_ps")
                    for dc in range(2):
                        for c in range(nk):
                            nc.tensor.matmul(
                                kv_ps[:, dc * D : (dc + 1) * D],
                                pnats[pi_][
                                    :, c * D + dc * 128 : c * D + dc * 128 + 128
                                ],
                                vmask[:, c * D : (c + 1) * D],
                                start=(c == 0),
                                stop=(c == nk - 1),
                            )
                    kv_sb = wp.tile([128, 512], mm_dt, tag=f"kv{pi_}", name="kv_sb")
                    nc.scalar.activation(
                        kv_sb, kv_ps, ACTF.Copy, scale=w_term[:, t : t + 1]
                    )
                    kvs.append((parts[pi_], kv_sb))

                for c in range(nk):
                    b = bank_of[c]
                    for pt, kv_sb in kvs:
                        for dc in range(2):
                            mm_count[b] += 1
                            nc.tensor.matmul(
                                outp[c],
                                pt[:, dc * M + c * 128 : dc * M + c * 128 + 128],
                                kv_sb[:, dc * D : (dc + 1) * D],
                                start=(mm_count[b] == 1),
                                stop=(mm_count[b] == mm_total[b]),
                                skip_group_check=True,
                            )

            T = len(terms)
            for t in range(T + 2):
                if t < T:
                    emit_B(t, terms[t])
                if 1 <= t < T + 1:
                    emit_C(t - 1, terms[t - 1])
                if t >= 2:
                    emit_D(t - 2, terms[t - 2])

            # ---------------- tail: mask + store (per chunk) ----------------
            o_sb = cp.tile([128, nk * D], F32)
            for c in range(nk):
                if c % 2 == 0:
                    nc.vector.tensor_scalar_mul(
                        o_sb[:, c * D : (c + 1) * D],
                        outp[c],
                        aux_sb[:, c_mask + c : c_mask + c + 1],
                    )
                else:
                    nc.scalar.activation(
                        o_sb[:, c * D : (c + 1) * D],
                        outp[c],
                        ACTF.Copy,
                        scale=aux_sb[:, c_mask + c : c_mask + c + 1],
                    )
                nc.sync.dma_start(
                    out=OUT[:, c * D : (c + 1) * D],
                    in_=o_sb[:, c * D : (c + 1) * D],
                )

    nc.finalize()
    return nc


_PROGRAM_CACHE = {}


def _get_program(nk, terms, G, J, mm_dt=F32R):
    key = (nk, tuple(terms), G, J, str(mm_dt))
    if key not in _PROGRAM_CACHE:
        _PROGRAM_CACHE[key] = _build_program(nk, terms, G, J, mm_dt)
    return _PROGRAM_CACHE[key]


def _find_terms(grid_u, grid_w):
    """Pair antipodal directions with equal weights; rest run as singles."""
    G = grid_u.shape[0]
    used = [False] * G
    terms = []
    for i in range(G):
        if used[i]:
            continue
        partner = -1
        for j in range(i + 1, G):
            if used[j]:
                continue
            if (
                np.allclose(grid_u[j], -grid_u[i], rtol=1e-6, atol=1e-7)
                and abs(float(grid_w[j]) - float(grid_w[i])) <= 1e-7
            ):
                partner = j
                break
        used[i] = True
        if partner >= 0:
            used[partner] = True
            terms.append((i, partner))
        else:
            terms.append((i,))
    return terms


def _prepare(inputs, positions, batch_segments, graph_mask, frequencies, grid_u,
             grid_w):
    n, p, s, f = inputs.shape
    d = p * s * f
    b = graph_mask.shape[0]
    G = grid_u.shape[0]
    J = frequencies.shape[0]
    assert d == 256 and f % 2 == 0 and b == N_CORES, (d, f, b)
    assert 2 * J == f, (J, f)

    x = np.asarray(inputs, np.float32).reshape(n, d)
    pos = np.asarray(positions, np.float32)
    seg = np.asarray(batch_segments)
    gmask = np.asarray(graph_mask)

    idxs = [np.nonzero(seg == c)[0] for c in range(b)]
    max_len = max(1, max(len(ix) for ix in idxs))
    nk = (max_len + 127) // 128
    M = 128 * nk

    terms = _find_terms(np.asarray(grid_u, np.float32), np.asarray(grid_w, np.float32))

    c_mask = 0
    c_post = nk
    c_ut = c_post + M
    c_frq = c_ut + G
    c_w = c_frq + J
    W = c_w + G

    in_maps = []
    for c in range(b):
        ix = idxs[c]
        pad = np.zeros(M, np.int64)
        pad[: len(ix)] = ix
        mask = np.zeros(M, np.float32)
        mask[: len(ix)] = gmask[seg[ix]].astype(np.float32)

        xs = x[pad]                       # (M, d)
        ps_ = pos[pad]                    # (M, 3)
        x_prep = np.ascontiguousarray(
            xs.reshape(nk, 128, d).transpose(1, 0, 2).reshape(128, nk * d)
        )
        aux = np.zeros((128, W), np.float32)
        aux[:, c_mask:c_mask + nk] = mask.reshape(nk, 128).T
        aux[0:3, c_post:c_post + M] = ps_.T
        aux[0:3, c_ut:c_ut + G] = np.asarray(grid_u, np.float32).T
        aux[0, c_frq:c_frq + J] = np.asarray(frequencies, np.float32)
        aux[0, c_w:c_w + G] = np.asarray(grid_w, np.float32)
        in_maps.append(dict(x=x_prep, aux=aux))

    meta = dict(n=n, p=p, s=s, f=f, d=d, b=b, G=G, J=J, nk=nk, M=M, idxs=idxs,
                terms=terms)
    return in_maps, meta


def _gather(results, meta, dtype):
    n, d, nk = meta["n"], meta["d"], meta["nk"]
    out = np.zeros((n, d), np.float32)
    for c, ix in enumerate(meta["idxs"]):
        o = results[c]["out"]                                  # (128, nk*d)
        o_nodes = o.reshape(128, nk, d).transpose(1, 0, 2).reshape(meta["M"], d)
        out[ix] = o_nodes[: len(ix)]
    return out.reshape(n, meta["p"], meta["s"], meta["f"]).astype(dtype)


def _run(inputs, positions, batch_segments, graph_mask, frequencies, grid_u,
         grid_w, trace=False, mm_dt=F32R):
    in_maps, meta = _prepare(inputs, positions, batch_segments, graph_mask,
                             frequencies, grid_u, grid_w)
    nc = _get_program(meta["nk"], meta["terms"], meta["G"], meta["J"], mm_dt)
    res = run_bass_kernel_spmd(
        nc, in_maps, core_ids=list(range(N_CORES)), trace=trace
    )
    out = _gather(res.results, meta, np.asarray(inputs).dtype)
    return out, res


def kernel(inputs, positions, batch_segments, graph_mask, frequencies, grid_u,
           grid_w):
    out, _ = _run(inputs, positions, batch_segments, graph_mask, frequencies,
                  grid_u, grid_w)
    return out


# revision 33
# speedup vs baseline: 1.0502x; 1.0502x over previous
"""EuclideanFastAttention Trainium2 kernel.

Full inputs -> shard graphs across 8 NeuronCores (1 graph/core) -> per-core
Bass/Tile kernel (Euclidean RoPE + linear attention over Lebedev quadrature)
-> gather full output.

Self-contained: hardcodes the problem geometry (N=2048, B=8, P=1, S=4, F=64,
G=14, J=32) but derives everything it can from the input arrays at runtime.
"""
import sys

sys.path.insert(0, "/opt/trn_rl_repo")

import numpy as np

import concourse.bacc as bacc
import concourse.bass as bass
import concourse.mybir as mybir
import concourse.tile as tile
from concourse import masks
from concourse.bass_utils import run_bass_kernel_spmd

F32 = mybir.dt.float32
F32R = mybir.dt.float32r
ACTF = mybir.ActivationFunctionType
ALU = mybir.AluOpType

PI = float(np.pi)
TWO_PI = float(2.0 * np.pi)
INV_2PI = float(1.0 / (2.0 * np.pi))
MAGIC = float(1.5 * 2.0**23)  # fp32 round-to-nearest-int magic constant

N_CORES = 8


def _bcast2(ap_2d, reps):
    """Read-broadcast a [P, M] AP to [P, reps, M] via a step-0 middle dim."""
    a = ap_2d
    return bass.AP(
        tensor=a.tensor,
        offset=a.offset,
        ap=[list(a.ap[0]), [0, reps], list(a.ap[1])],
    )


def _build_program(nk, terms, G, J, mm_dt=F32R):
    """Build the SPMD per-core program.

    nk:    number of 128-node chunks per core (M_cap = 128*nk)
    terms: list of tuples; (i, j) = antipodal pair (u_j = -u_i, w_j = w_i),
           (s,) = single grid direction.
    """
    D = 256
    M = 128 * nk
    # aux blob column layout
    c_mask = 0
    c_post = nk
    c_ut = c_post + M
    c_frq = c_ut + G
    c_w = c_frq + J
    W = c_w + G

    nc = bacc.Bacc()
    X = nc.declare_dram_parameter("x", [128, nk * D], F32, isOutput=False)
    AUX = nc.declare_dram_parameter("aux", [128, W], F32, isOutput=False)
    OUT = nc.declare_dram_parameter("out", [128, nk * D], F32, isOutput=True)

    with tile.TileContext(nc) as tc:
        with (
            tc.tile_pool(name="const", bufs=1) as cp,
            tc.tile_pool(name="work", bufs=4) as wp,
            tc.tile_pool(name="trps", bufs=4, space="PSUM") as trps,
            tc.tile_pool(name="kvps", bufs=(3 if nk <= 2 else 2), space="PSUM") as kvps,
            tc.tile_pool(name="outps", bufs=1, space="PSUM") as outps,
        ):
            # ---------------- setup ----------------
            x_sb = cp.tile([128, nk * D], F32)
            aux_sb = cp.tile([128, W], F32)
            # rows 0:3 of cols nk.. carry posT/uT/freq/grid_w -> the whole
            # angle-pipeline prefix; land them first in a tiny DMA
            nc.sync.dma_start(out=aux_sb[0:3, c_post:W], in_=AUX[0:3, c_post:W])
            nc.sync.dma_start(out=aux_sb[:, 0:nk], in_=AUX[:, 0:nk])
            nc.sync.dma_start(out=x_sb, in_=X[:, :])

            ident = cp.tile([128, 128], F32)
            masks.make_identity(nc, ident[:])
            ident_r = cp.tile([128, 128], F32R)
            nc.vector.tensor_copy(ident_r, ident)

            zero_col = cp.tile([128, 1], F32)
            nc.vector.memset(zero_col, 0.0)
            halfpi_col = cp.tile([128, 1], F32)
            nc.vector.memset(halfpi_col, PI / 2)

            # dotposT[g, n] = sum_c uT[c, g] * posT[c, n]
            dp_ps = trps.tile([G, M], F32, tag="tp")
            nc.tensor.matmul(
                dp_ps,
                aux_sb[0:3, c_ut : c_ut + G],
                aux_sb[0:3, c_post : c_post + M],
                start=True,
                stop=True,
            )
            dp_sb = cp.tile([G, M], F32)
            nc.vector.tensor_copy(dp_sb, dp_ps)
            # collapse [G, M] partitions into one row (partition_broadcast
            # can only read partition 0)
            dp_row = cp.tile([1, G * M], F32)
            nc.sync.dma_start(out=dp_row[0:1, :], in_=dp_sb[:, :])

            # freq_col[p] = freq[(p % 64)//2] / (2*pi), via row build + matmul
            frow = cp.tile([1, 128], F32)
            fv = frow[:].rearrange("p (a j two) -> p a j two", a=2, two=2)
            for a in range(2):
                for t in range(2):
                    nc.vector.tensor_copy(
                        fv[0:1, a, :, t], aux_sb[0:1, c_frq : c_frq + J]
                    )
            ones1 = cp.tile([1, 1], F32)
            nc.vector.memset(ones1, 1.0)
            fc_ps = trps.tile([128, 1], F32, tag="tp")
            nc.tensor.matmul(fc_ps, frow, ones1, start=True, stop=True)
            f2pi_col = cp.tile([128, 1], F32)
            nc.vector.tensor_scalar_mul(f2pi_col, fc_ps, INV_2PI)  # psum read: DVE

            # grid weights broadcast; per-term weight columns
            w_bc = cp.tile([128, G], F32)
            nc.gpsimd.partition_broadcast(w_bc, aux_sb[0:1, c_w : c_w + G])
            w_term = cp.tile([128, len(terms)], F32)
            for t, term in enumerate(terms):
                if len(term) == 2:
                    i, j = term
                    nc.gpsimd.tensor_add(
                        w_term[:, t : t + 1], w_bc[:, i : i + 1], w_bc[:, j : j + 1]
                    )
                else:
                    (s,) = term
                    nc.gpsimd.tensor_copy(w_term[:, t : t + 1], w_bc[:, s : s + 1])

            # xswap natural: xsw[:, 2q] = -x[:, 2q+1], xsw[:, 2q+1] = x[:, 2q]
            xsw = cp.tile([128, nk * D], F32)
            xv = x_sb[:].rearrange("p (q two) -> p q two", two=2)
            sv = xsw[:].rearrange("p (q two) -> p q two", two=2)
            nc.scalar.activation(sv[:, :, 0:1], xv[:, :, 1:2], ACTF.Copy, scale=-1.0)
            nc.scalar.activation(sv[:, :, 1:2], xv[:, :, 0:1], ACTF.Copy)

            # transposed copies into one tile: xTc[:, 0:2M] = xT (x transposed,
            # [dchunk partition, n free]); xTc[:, 2M:4M] = xswT (xsw transposed)
            xTc = cp.tile([128, 4 * M], F32)
            for si, src in enumerate((x_sb, xsw)):
                tp = trps.tile([128, 512], F32, tag="tp", name="tp_setup")
                for c in range(nk):
                    for dc in range(2):
                        nc.tensor.transpose(
                            tp[:, (c * 2 + dc) * 128 : (c * 2 + dc) * 128 + 128],
                            src[:, c * D + dc * 128 : c * D + dc * 128 + 128],
                            ident,
                        )
                # single permuting copy: (c dc l) -> (dc c l)
                nc.scalar.activation(
                    xTc[:, si * 2 * M : (si + 1) * 2 * M].rearrange(
                        "p (dc c l) -> p dc c l", dc=2, c=nk
                    ),
                    tp[:, : nk * 256].rearrange("p (c dc l) -> p dc c l", c=nk, dc=2),
                    ACTF.Copy,
                )

            # v = x * mask, rounded to matmul dtype
            vmask = cp.tile([128, nk * D], mm_dt)
            for c in range(nk):
                nc.scalar.activation(
                    vmask[:, c * D : (c + 1) * D],
                    x_sb[:, c * D : (c + 1) * D],
                    ACTF.Copy,
                    scale=aux_sb[:, c_mask + c : c_mask + c + 1],
                )

            # persistent output accumulators packed into one PSUM bank;
            # per-chunk groups interleave in one zero region, so the group
            # check is skipped on the out matmuls (has_written is per element)
            out_bank = outps.tile([128, nk * D], F32, tag="out_bank",
                                  name="out_bank")
            outp = [out_bank[:, c * D : (c + 1) * D] for c in range(nk)]
            # start=True zeroes the whole PSUM zero region (one 2KB bank =
            # two 256-col chunks), so exactly the first matmul into each bank
            # carries start=True and the last one carries stop=True
            n_banks = (nk + 1) // 2
            bank_of = [c // 2 for c in range(nk)]
            mm_count = [0] * n_banks
            mm_total = [0] * n_banks
            for term in terms:
                for c in range(nk):
                    mm_total[bank_of[c]] += 4 if len(term) == 2 else 2

            # ---- software-pipelined main loop: B(t) | C(t-1) | D(t-2) ----
            # B: angles + fused sin/cos   (Pool/DVE + ACT)
            # C: fused rotation t12       (DVE)
            # D: transposes + kv + out    (PE + DVE/ACT psum copies)
            sc_t = {}
            t12_t = {}
            parts_t = {}

            def emit_B(t, term):
                i = term[0]
                dpb = wp.tile([128, M], F32, tag="dpb", name="dpb")
                if i == 0:
                    # row 0 lives on partition 0 of dp_sb already; skip the
                    # dp_row collapse dependency for the first term
                    nc.gpsimd.partition_broadcast(dpb, dp_sb[0:1, :])
                else:
                    nc.gpsimd.partition_broadcast(
                        dpb, dp_row[0:1, i * M : (i + 1) * M]
                    )
                eng = nc.gpsimd if t % 2 == 0 else nc.vector
                eng2 = eng
                w_t = wp.tile([128, M], F32, tag="w_t", name="w_t")
                eng.tensor_scalar_mul(w_t, dpb, f2pi_col[:, 0:1])
                k_sc = wp.tile([128, 2 * M], F32, tag="k_sc", name="k_sc")
                eng2.tensor_scalar(
                    k_sc[:, 0:M], w_t, MAGIC, MAGIC, ALU.add, ALU.subtract
                )
                t_c = wp.tile([128, M], F32, tag="t_c", name="t_c")
                eng.tensor_scalar(t_c, w_t, 0.25, MAGIC, ALU.add, ALU.add)
                eng.tensor_scalar(
                    k_sc[:, M : 2 * M], t_c, MAGIC, 0.25, ALU.subtract,
                    ALU.subtract,
                )
                d_sc = wp.tile([128, 2 * M], F32, tag="d_sc", name="d_sc")
                (nc.gpsimd if t % 2 == 0 else nc.vector).tensor_sub(
                    d_sc[:].rearrange("p (two m) -> p two m", two=2),
                    _bcast2(w_t[:], 2),
                    k_sc[:].rearrange("p (two m) -> p two m", two=2),
                )
                sc = wp.tile([128, 2 * M], F32, tag="sc", name="sc")
                nc.scalar.activation(
                    sc, d_sc, ACTF.Sin, bias=zero_col[:, 0:1], scale=TWO_PI
                )
                sc_t[t] = sc

            def emit_C(t, term):
                t12 = wp.tile([128, 4 * M], mm_dt, tag="t12", name="t12")
                sc_ap = sc_t.pop(t)[:]
                if t == 0:
                    # split so part0's transposes can start one op earlier
                    nc.vector.tensor_mul(
                        t12[:, 0 : 2 * M].rearrange("p (dc m) -> p dc m", dc=2),
                        xTc[:, 0 : 2 * M].rearrange("p (dc m) -> p dc m", dc=2),
                        _bcast2(sc_ap[:, M : 2 * M], 2),
                    )
                    nc.vector.tensor_mul(
                        t12[:, 2 * M : 4 * M].rearrange("p (dc m) -> p dc m", dc=2),
                        xTc[:, 2 * M : 4 * M].rearrange("p (dc m) -> p dc m", dc=2),
                        _bcast2(sc_ap[:, 0:M], 2),
                    )
                else:
                    sc_in = bass.AP(
                        tensor=sc_ap.tensor,
                        offset=sc_ap.offset + M,
                        ap=[list(sc_ap.ap[0]), [-M, 2], [0, 2], [1, M]],
                    )
                    nc.vector.tensor_mul(
                        t12[:].rearrange("p (h dc m) -> p h dc m", h=2, dc=2),
                        xTc[:].rearrange("p (h dc m) -> p h dc m", h=2, dc=2),
                        sc_in,
                    )
                t12_t[t] = t12
                if len(term) == 2:
                    parts_t[t] = [t12[:, 0 : 2 * M], t12[:, 2 * M : 4 * M]]
                else:
                    rT = wp.tile([128, 2 * M], mm_dt, tag="rT", name="rT")
                    nc.vector.tensor_add(
                        rT, t12[:, 0 : 2 * M], t12[:, 2 * M : 4 * M]
                    )
                    parts_t[t] = [rT]

            def emit_D(t, term):
                parts = parts_t.pop(t)
                t12_t.pop(t, None)
                np_ = len(parts)
                pnats = []
                for pi_, pt in enumerate(parts):
                    # per-part PSUM tile so each pnat copy gates on its own
                    # 4 transposes, not all 8
                    tp = trps.tile([128, 512], mm_dt, tag="tp", name="tp_main")
                    for c in range(nk):
                        for dc in range(2):
                            col = c * D + dc * 128
                            nc.tensor.matmul(
                                tp[:, col : col + 128],
                                pt[:, dc * M + c * 128 : dc * M + c * 128 + 128],
                                ident_r,
                                is_transpose=True,
                            )
                    pnat = wp.tile([128, nk * D], mm_dt, tag=f"nat{pi_}",
                                   name="pnat")
                    if pi_ == 0:
                        nc.scalar.activation(pnat, tp[:, : nk * D], ACTF.Copy)
                    else:
                        nc.vector.tensor_copy(pnat, tp[:, : nk * D])
                    pnats.append(pnat)

                kvs = []
                for pi_ in range(np_):
                    kv_ps = kvps.tile([128, 512], F32, tag="kv_ps", name="kv_ps")
                    for dc in range(2):
                        for c in range(nk):
                            nc.tensor.matmul(
                                kv_ps[:, dc * D : (dc + 1) * D],
                                pnats[pi_][
                                    :, c * D + dc * 128 : c * D + dc * 128 + 128
                                ],
                                vmask[:, c * D : (c + 1) * D],
                                start=(c == 0),
                                stop=(c == nk - 1),
                            )
                    kv_sb = wp.tile([128, 512], mm_dt, tag=f"kv{pi_}", name="kv_sb")
                    nc.scalar.activation(
                        kv_sb, kv_ps, ACTF.Copy, scale=w_term[:, t : t + 1]
                    )
                    kvs.append((parts[pi_], kv_sb))

                for c in range(nk):
                    b = bank_of[c]
                    for pt, kv_sb in kvs:
                        for dc in range(2):
                            mm_count[b] += 1
                            nc.tensor.matmul(
                                outp[c],
                                pt[:, dc * M + c * 128 : dc * M + c * 128 + 128],
                                kv_sb[:, dc * D : (dc + 1) * D],
                                start=(mm_count[b] == 1),
                                stop=(mm_count[b] == mm_total[b]),
                                skip_group_check=True,
                            )

            T = len(terms)
            for t in range(T + 2):
                if t < T:
                    emit_B(t, terms[t])
                if 1 <= t < T + 1:
                    emit_C(t - 1, terms[t - 1])
                if t >= 2:
                    emit_D(t - 2, terms[t - 2])

            # ---------------- tail: mask + store (per chunk) ----------------
            o_sb = cp.tile([128, nk * D], F32)
            for c in range(nk):
                if c % 2 == 0:
                    nc.vector.tensor_scalar_mul(
                        o_sb[:, c * D : (c + 1) * D],
                        outp[c],
                        aux_sb[:, c_mask + c : c_mask + c + 1],
                    )
                else:
                    nc.scalar.activation(
                        o_sb[:, c * D : (c + 1) * D],
                        outp[c],
                        ACTF.Copy,
                        scale=aux_sb[:, c_mask + c : c_mask + c + 1],
                    )
                nc.sync.dma_start(
                    out=OUT[:, c * D : (c + 1) * D],
                    in_=o_sb[:, c * D : (c + 1) * D],
                )

    nc.finalize()
    return nc


_PROGRAM_CACHE = {}


def _get_program(nk, terms, G, J, mm_dt=F32R):
    key = (nk, tuple(terms), G, J, str(mm_dt))
    if key not in _PROGRAM_CACHE:
        _PROGRAM_CACHE[key] = _build_program(nk, terms, G, J, mm_dt)
    return _PROGRAM_CACHE[key]


def _find_terms(grid_u, grid_w):
    """Pair antipodal directions with equal weights; rest run as singles."""
    G = grid_u.shape[0]
    used = [False] * G
    terms = []
    for i in range(G):
        if used[i]:
            continue
        partner = -1
        for j in range(i + 1, G):
            if used[j]:
                continue
            if (
                np.allclose(grid_u[j], -grid_u[i], rtol=1e-6, atol=1e-7)
                and abs(float(grid_w[j]) - float(grid_w[i])) <= 1e-7
            ):
                partner = j
                break
        used[i] = True
        if partner >= 0:
            used[partner] = True
            terms.append((i, partner))
        else:
            terms.append((i,))
    return terms


def _prepare(inputs, positions, batch_segments, graph_mask, frequencies, grid_u,
             grid_w):
    n, p, s, f = inputs.shape
    d = p * s * f
    b = graph_mask.shape[0]
    G = grid_u.shape[0]
    J = frequencies.shape[0]
    assert d == 256 and f % 2 == 0 and b == N_CORES, (d, f, b)
    assert 2 * J == f, (J, f)

    x = np.asarray(inputs, np.float32).reshape(n, d)
    pos = np.asarray(positions, np.float32)
    seg = np.asarray(batch_segments)
    gmask = np.asarray(graph_mask)

    idxs = [np.nonzero(seg == c)[0] for c in range(b)]
    max_len = max(1, max(len(ix) for ix in idxs))
    nk = (max_len + 127) // 128
    M = 128 * nk

    terms = _find_terms(np.asarray(grid_u, np.float32), np.asarray(grid_w, np.float32))

    c_mask = 0
    c_post = nk
    c_ut = c_post + M
    c_frq = c_ut + G
    c_w = c_frq + J
    W = c_w + G

    in_maps = []
    for c in range(b):
        ix = idxs[c]
        pad = np.zeros(M, np.int64)
        pad[: len(ix)] = ix
        mask = np.zeros(M, np.float32)
        mask[: len(ix)] = gmask[seg[ix]].astype(np.float32)

        xs = x[pad]                       # (M, d)
        ps_ = pos[pad]                    # (M, 3)
        x_prep = np.ascontiguousarray(
            xs.reshape(nk, 128, d).transpose(1, 0, 2).reshape(128, nk * d)
        )
        aux = np.zeros((128, W), np.float32)
        aux[:, c_mask:c_mask + nk] = mask.reshape(nk, 128).T
        aux[0:3, c_post:c_post + M] = ps_.T
        aux[0:3, c_ut:c_ut + G] = np.asarray(grid_u, np.float32).T
        aux[0, c_frq:c_frq + J] = np.asarray(frequencies, np.float32)
        aux[0, c_w:c_w + G] = np.asarray(grid_w, np.float32)
        in_maps.append(dict(x=x_prep, aux=aux))

    meta = dict(n=n, p=p, s=s, f=f, d=d, b=b, G=G, J=J, nk=nk, M=M, idxs=idxs,
                terms=terms)
    return in_maps, meta


def _gather(results, meta, dtype):
    n, d, nk = meta["n"], meta["d"], meta["nk"]
    out = np.zeros((n, d), np.float32)
    for c, ix in enumerate(meta["idxs"]):
        o = results[c]["out"]                                  # (128, nk*d)
        o_nodes = o.reshape(128, nk, d).transpose(1, 0, 2).reshape(meta["M"], d)
        out[ix] = o_nodes[: len(ix)]
    return out.reshape(n, meta["p"], meta["s"], meta["f"]).astype(dtype)


def _run(inputs, positions, batch_segments, graph_mask, frequencies, grid_u,
         grid_w, trace=False, mm_dt=F32R):
    in_maps, meta = _prepare(inputs, positions, batch_segments, graph_mask,
                             frequencies, grid_u, grid_w)
    nc = _get_program(meta["nk"], meta["terms"], meta["G"], meta["J"], mm_dt)
    res = run_bass_kernel_spmd(
        nc, in_maps, core_ids=list(range(N_CORES)), trace=trace
    )
    out = _gather(res.results, meta, np.asarray(inputs).dtype)
    return out, res


def kernel(inputs, positions, batch_segments, graph_mask, frequencies, grid_u,
           grid_w):
    out, _ = _run(inputs, positions, batch_segments, graph_mask, frequencies,
                  grid_u, grid_w)
    return out


# revision 34
# speedup vs baseline: 1.0693x; 1.0182x over previous
"""EuclideanFastAttention Trainium2 kernel.

Full inputs -> shard graphs across 8 NeuronCores (1 graph/core) -> per-core
Bass/Tile kernel (Euclidean RoPE + linear attention over Lebedev quadrature)
-> gather full output.

Self-contained: hardcodes the problem geometry (N=2048, B=8, P=1, S=4, F=64,
G=14, J=32) but derives everything it can from the input arrays at runtime.
"""
import sys

sys.path.insert(0, "/opt/trn_rl_repo")

import numpy as np

import concourse.bacc as bacc
import concourse.bass as bass
import concourse.mybir as mybir
import concourse.tile as tile
from concourse import masks
from concourse.bass_utils import run_bass_kernel_spmd

F32 = mybir.dt.float32
F32R = mybir.dt.float32r
ACTF = mybir.ActivationFunctionType
ALU = mybir.AluOpType

PI = float(np.pi)
TWO_PI = float(2.0 * np.pi)
INV_2PI = float(1.0 / (2.0 * np.pi))
MAGIC = float(1.5 * 2.0**23)  # fp32 round-to-nearest-int magic constant

N_CORES = 8


def _bcast2(ap_2d, reps):
    """Read-broadcast a [P, M] AP to [P, reps, M] via a step-0 middle dim."""
    a = ap_2d
    return bass.AP(
        tensor=a.tensor,
        offset=a.offset,
        ap=[list(a.ap[0]), [0, reps], list(a.ap[1])],
    )


def _build_program(nk, terms, G, J, mm_dt=F32R):
    """Build the SPMD per-core program.

    nk:    number of 128-node chunks per core (M_cap = 128*nk)
    terms: list of tuples; (i, j) = antipodal pair (u_j = -u_i, w_j = w_i),
           (s,) = single grid direction.
    """
    D = 256
    M = 128 * nk
    # aux blob column layout
    c_mask = 0
    c_post = nk
    c_ut = c_post + M
    c_frq = c_ut + G
    c_w = c_frq + J
    W = c_w + G

    nc = bacc.Bacc()
    X = nc.declare_dram_parameter("x", [128, nk * D], F32, isOutput=False)
    AUX = nc.declare_dram_parameter("aux", [128, W], F32, isOutput=False)
    OUT = nc.declare_dram_parameter("out", [128, nk * D], F32, isOutput=True)

    with tile.TileContext(nc) as tc:
        with (
            tc.tile_pool(name="const", bufs=1) as cp,
            tc.tile_pool(name="work", bufs=4) as wp,
            tc.tile_pool(name="trps", bufs=4, space="PSUM") as trps,
            tc.tile_pool(name="kvps", bufs=(3 if nk <= 2 else 2), space="PSUM") as kvps,
            tc.tile_pool(name="outps", bufs=1, space="PSUM") as outps,
        ):
            # ---------------- setup ----------------
            x_sb = cp.tile([128, nk * D], F32)
            aux_sb = cp.tile([128, W], F32)
            # rows 0:3 of cols nk.. carry posT/uT/freq/grid_w -> the whole
            # angle-pipeline prefix; land them first in a tiny DMA
            nc.sync.dma_start(out=aux_sb[0:3, c_post:W], in_=AUX[0:3, c_post:W])
            nc.sync.dma_start(out=aux_sb[:, 0:nk], in_=AUX[:, 0:nk])
            nc.sync.dma_start(out=x_sb, in_=X[:, :])

            zero_col = cp.tile([128, 1], F32)
            nc.vector.memset(zero_col, 0.0)
            halfpi_col = cp.tile([128, 1], F32)
            nc.vector.memset(halfpi_col, PI / 2)

            # dotposT[g, n] = sum_c uT[c, g] * posT[c, n]
            dp_ps = trps.tile([G, M], F32, tag="tp")
            nc.tensor.matmul(
                dp_ps,
                aux_sb[0:3, c_ut : c_ut + G],
                aux_sb[0:3, c_post : c_post + M],
                start=True,
                stop=True,
            )
            dp_sb = cp.tile([G, M], F32)
            nc.vector.tensor_copy(dp_sb, dp_ps)
            # collapse [G, M] partitions into one row (partition_broadcast
            # can only read partition 0)
            dp_row = cp.tile([1, G * M], F32)
            nc.sync.dma_start(out=dp_row[0:1, :], in_=dp_sb[:, :])

            # freq_col[p] = freq[(p % 64)//2] / (2*pi), via row build + matmul
            frow = cp.tile([1, 128], F32)
            fv = frow[:].rearrange("p (a j two) -> p a j two", a=2, two=2)
            for a in range(2):
                for t in range(2):
                    nc.vector.tensor_copy(
                        fv[0:1, a, :, t], aux_sb[0:1, c_frq : c_frq + J]
                    )
            ones1 = cp.tile([1, 1], F32)
            nc.vector.memset(ones1, 1.0)
            fc_ps = trps.tile([128, 1], F32, tag="tp")
            nc.tensor.matmul(fc_ps, frow, ones1, start=True, stop=True)
            f2pi_col = cp.tile([128, 1], F32)
            nc.vector.tensor_scalar_mul(f2pi_col, fc_ps, INV_2PI)  # psum read: DVE

            # persistent output accumulators packed into one PSUM bank;
            # per-chunk groups interleave in one zero region, so the group
            # check is skipped on the out matmuls (has_written is per element)
            out_bank = outps.tile([128, nk * D], F32, tag="out_bank",
                                  name="out_bank")
            outp = [out_bank[:, c * D : (c + 1) * D] for c in range(nk)]
            # start=True zeroes the whole PSUM zero region (one 2KB bank =
            # two 256-col chunks), so exactly the first matmul into each bank
            # carries start=True and the last one carries stop=True
            n_banks = (nk + 1) // 2
            bank_of = [c // 2 for c in range(nk)]
            mm_count = [0] * n_banks
            mm_total = [0] * n_banks
            for term in terms:
                for c in range(nk):
                    mm_total[bank_of[c]] += 4 if len(term) == 2 else 2

            # ---- software-pipelined main loop: B(t) | C(t-1) | D(t-2) ----
            # B: angles + fused sin/cos   (Pool/DVE + ACT)
            # C: fused rotation t12       (DVE)
            # D: transposes + kv + out    (PE + DVE/ACT psum copies)
            sc_t = {}
            t12_t = {}
            parts_t = {}

            def emit_B(t, term):
                i = term[0]
                dpb = wp.tile([128, M], F32, tag="dpb", name="dpb")
                if i == 0:
                    # row 0 lives on partition 0 of dp_sb already; skip the
                    # dp_row collapse dependency for the first term
                    nc.gpsimd.partition_broadcast(dpb, dp_sb[0:1, :])
                else:
                    nc.gpsimd.partition_broadcast(
                        dpb, dp_row[0:1, i * M : (i + 1) * M]
                    )
                eng = nc.gpsimd if t % 2 == 0 else nc.vector
                eng2 = eng
                w_t = wp.tile([128, M], F32, tag="w_t", name="w_t")
                eng.tensor_scalar_mul(w_t, dpb, f2pi_col[:, 0:1])
                k_sc = wp.tile([128, 2 * M], F32, tag="k_sc", name="k_sc")
                eng2.tensor_scalar(
                    k_sc[:, 0:M], w_t, MAGIC, MAGIC, ALU.add, ALU.subtract
                )
                t_c = wp.tile([128, M], F32, tag="t_c", name="t_c")
                eng.tensor_scalar(t_c, w_t, 0.25, MAGIC, ALU.add, ALU.add)
                eng.tensor_scalar(
                    k_sc[:, M : 2 * M], t_c, MAGIC, 0.25, ALU.subtract,
                    ALU.subtract,
                )
                d_sc = wp.tile([128, 2 * M], F32, tag="d_sc", name="d_sc")
                (nc.gpsimd if t % 2 == 0 else nc.vector).tensor_sub(
                    d_sc[:].rearrange("p (two m) -> p two m", two=2),
                    _bcast2(w_t[:], 2),
                    k_sc[:].rearrange("p (two m) -> p two m", two=2),
                )
                sc = wp.tile([128, 2 * M], F32, tag="sc", name="sc")
                nc.scalar.activation(
                    sc, d_sc, ACTF.Sin, bias=zero_col[:, 0:1], scale=TWO_PI
                )
                sc_t[t] = sc

            def emit_C(t, term):
                t12 = wp.tile([128, 4 * M], mm_dt, tag="t12", name="t12")
                sc_ap = sc_t.pop(t)[:]
                if t == 0:
                    # split so part0's transposes can start one op earlier
                    nc.vector.tensor_mul(
                        t12[:, 0 : 2 * M].rearrange("p (dc m) -> p dc m", dc=2),
                        xTc[:, 0 : 2 * M].rearrange("p (dc m) -> p dc m", dc=2),
                        _bcast2(sc_ap[:, M : 2 * M], 2),
                    )
                    nc.vector.tensor_mul(
                        t12[:, 2 * M : 4 * M].rearrange("p (dc m) -> p dc m", dc=2),
                        xTc[:, 2 * M : 4 * M].rearrange("p (dc m) -> p dc m", dc=2),
                        _bcast2(sc_ap[:, 0:M], 2),
                    )
                else:
                    sc_in = bass.AP(
                        tensor=sc_ap.tensor,
                        offset=sc_ap.offset + M,
                        ap=[list(sc_ap.ap[0]), [-M, 2], [0, 2], [1, M]],
                    )
                    nc.vector.tensor_mul(
                        t12[:].rearrange("p (h dc m) -> p h dc m", h=2, dc=2),
                        xTc[:].rearrange("p (h dc m) -> p h dc m", h=2, dc=2),
                        sc_in,
                    )
                t12_t[t] = t12
                if len(term) == 2:
                    parts_t[t] = [t12[:, 0 : 2 * M], t12[:, 2 * M : 4 * M]]
                else:
                    rT = wp.tile([128, 2 * M], mm_dt, tag="rT", name="rT")
                    nc.vector.tensor_add(
                        rT, t12[:, 0 : 2 * M], t12[:, 2 * M : 4 * M]
                    )
                    parts_t[t] = [rT]

            def emit_D(t, term):
                parts = parts_t.pop(t)
                t12_t.pop(t, None)
                np_ = len(parts)
                pnats = []
                for pi_, pt in enumerate(parts):
                    # per-part PSUM tile so each pnat copy gates on its own
                    # 4 transposes, not all 8
                    tp = trps.tile([128, 512], mm_dt, tag="tp", name="tp_main")
                    for c in range(nk):
                        for dc in range(2):
                            col = c * D + dc * 128
                            nc.tensor.matmul(
                                tp[:, col : col + 128],
                                pt[:, dc * M + c * 128 : dc * M + c * 128 + 128],
                                ident_r,
                                is_transpose=True,
                            )
                    pnat = wp.tile([128, nk * D], mm_dt, tag=f"nat{pi_}",
                                   name="pnat")
                    if pi_ == 0:
                        nc.scalar.activation(pnat, tp[:, : nk * D], ACTF.Copy)
                    else:
                        nc.vector.tensor_copy(pnat, tp[:, : nk * D])
                    pnats.append(pnat)

                kvs = []
                for pi_ in range(np_):
                    kv_ps = kvps.tile([128, 512], F32, tag="kv_ps", name="kv_ps")
                    for dc in range(2):
                        for c in range(nk):
                            nc.tensor.matmul(
                                kv_ps[:, dc * D : (dc + 1) * D],
                                pnats[pi_][
                                    :, c * D + dc * 128 : c * D + dc * 128 + 128
                                ],
                                vmask[:, c * D : (c + 1) * D],
                                start=(c == 0),
                                stop=(c == nk - 1),
                            )
                    kv_sb = wp.tile([128, 512], mm_dt, tag=f"kv{pi_}", name="kv_sb")
                    nc.scalar.activation(
                        kv_sb, kv_ps, ACTF.Copy, scale=w_term[:, t : t + 1]
                    )
                    kvs.append((parts[pi_], kv_sb))

                for c in range(nk):
                    b = bank_of[c]
                    for pt, kv_sb in kvs:
                        for dc in range(2):
                            mm_count[b] += 1
                            nc.tensor.matmul(
                                outp[c],
                                pt[:, dc * M + c * 128 : dc * M + c * 128 + 128],
                                kv_sb[:, dc * D : (dc + 1) * D],
                                start=(mm_count[b] == 1),
                                stop=(mm_count[b] == mm_total[b]),
                                skip_group_check=True,
                            )

            # critical prefix first: B(0) before the bulk setup so Pool/DVE
            # start the angle chain immediately; setup fills the gap
            emit_B(0, terms[0])
            ident = cp.tile([128, 128], F32)
            masks.make_identity(nc, ident[:])
            ident_r = cp.tile([128, 128], F32R)
            nc.vector.tensor_copy(ident_r, ident)

            # grid weights broadcast; per-term weight columns
            w_bc = cp.tile([128, G], F32)
            nc.gpsimd.partition_broadcast(w_bc, aux_sb[0:1, c_w : c_w + G])
            w_term = cp.tile([128, len(terms)], F32)
            for t, term in enumerate(terms):
                if len(term) == 2:
                    i, j = term
                    nc.gpsimd.tensor_add(
                        w_term[:, t : t + 1], w_bc[:, i : i + 1], w_bc[:, j : j + 1]
                    )
                else:
                    (s,) = term
                    nc.gpsimd.tensor_copy(w_term[:, t : t + 1], w_bc[:, s : s + 1])

            # xswap natural: xsw[:, 2q] = -x[:, 2q+1], xsw[:, 2q+1] = x[:, 2q]
            xsw = cp.tile([128, nk * D], F32)
            xv = x_sb[:].rearrange("p (q two) -> p q two", two=2)
            sv = xsw[:].rearrange("p (q two) -> p q two", two=2)
            nc.scalar.activation(sv[:, :, 0:1], xv[:, :, 1:2], ACTF.Copy, scale=-1.0)
            nc.scalar.activation(sv[:, :, 1:2], xv[:, :, 0:1], ACTF.Copy)

            # transposed copies into one tile: xTc[:, 0:2M] = xT (x transposed,
            # [dchunk partition, n free]); xTc[:, 2M:4M] = xswT (xsw transposed)
            xTc = cp.tile([128, 4 * M], F32)
            for si, src in enumerate((x_sb, xsw)):
                tp = trps.tile([128, 512], F32, tag="tp", name="tp_setup")
                for c in range(nk):
                    for dc in range(2):
                        nc.tensor.transpose(
                            tp[:, (c * 2 + dc) * 128 : (c * 2 + dc) * 128 + 128],
                            src[:, c * D + dc * 128 : c * D + dc * 128 + 128],
                            ident,
                        )
                # single permuting copy: (c dc l) -> (dc c l)
                nc.scalar.activation(
                    xTc[:, si * 2 * M : (si + 1) * 2 * M].rearrange(
                        "p (dc c l) -> p dc c l", dc=2, c=nk
                    ),
                    tp[:, : nk * 256].rearrange("p (c dc l) -> p dc c l", c=nk, dc=2),
                    ACTF.Copy,
                )

            # v = x * mask, rounded to matmul dtype
            vmask = cp.tile([128, nk * D], mm_dt)
            for c in range(nk):
                nc.scalar.activation(
                    vmask[:, c * D : (c + 1) * D],
                    x_sb[:, c * D : (c + 1) * D],
                    ACTF.Copy,
                    scale=aux_sb[:, c_mask + c : c_mask + c + 1],
                )


            T = len(terms)
            for t in range(1, T + 2):
                if t < T:
                    emit_B(t, terms[t])
                if t <= T:
                    emit_C(t - 1, terms[t - 1])
                if t >= 2:
                    emit_D(t - 2, terms[t - 2])

            # ---------------- tail: mask + store (per chunk) ----------------
            o_sb = cp.tile([128, nk * D], F32)
            for c in range(nk):
                if c % 2 == 0:
                    nc.vector.tensor_scalar_mul(
                        o_sb[:, c * D : (c + 1) * D],
                        outp[c],
                        aux_sb[:, c_mask + c : c_mask + c + 1],
                    )
                else:
                    nc.scalar.activation(
                        o_sb[:, c * D : (c + 1) * D],
                        outp[c],
                        ACTF.Copy,
                        scale=aux_sb[:, c_mask + c : c_mask + c + 1],
                    )
                nc.sync.dma_start(
                    out=OUT[:, c * D : (c + 1) * D],
                    in_=o_sb[:, c * D : (c + 1) * D],
                )

    nc.finalize()
    return nc


_PROGRAM_CACHE = {}


def _get_program(nk, terms, G, J, mm_dt=F32R):
    key = (nk, tuple(terms), G, J, str(mm_dt))
    if key not in _PROGRAM_CACHE:
        _PROGRAM_CACHE[key] = _build_program(nk, terms, G, J, mm_dt)
    return _PROGRAM_CACHE[key]


def _find_terms(grid_u, grid_w):
    """Pair antipodal directions with equal weights; rest run as singles."""
    G = grid_u.shape[0]
    used = [False] * G
    terms = []
    for i in range(G):
        if used[i]:
            continue
        partner = -1
        for j in range(i + 1, G):
            if used[j]:
                continue
            if (
                np.allclose(grid_u[j], -grid_u[i], rtol=1e-6, atol=1e-7)
                and abs(float(grid_w[j]) - float(grid_w[i])) <= 1e-7
            ):
                partner = j
                break
        used[i] = True
        if partner >= 0:
            used[partner] = True
            terms.append((i, partner))
        else:
            terms.append((i,))
    return terms


def _prepare(inputs, positions, batch_segments, graph_mask, frequencies, grid_u,
             grid_w):
    n, p, s, f = inputs.shape
    d = p * s * f
    b = graph_mask.shape[0]
    G = grid_u.shape[0]
    J = frequencies.shape[0]
    assert d == 256 and f % 2 == 0 and b == N_CORES, (d, f, b)
    assert 2 * J == f, (J, f)

    x = np.asarray(inputs, np.float32).reshape(n, d)
    pos = np.asarray(positions, np.float32)
    seg = np.asarray(batch_segments)
    gmask = np.asarray(graph_mask)

    idxs = [np.nonzero(seg == c)[0] for c in range(b)]
    max_len = max(1, max(len(ix) for ix in idxs))
    nk = (max_len + 127) // 128
    M = 128 * nk

    terms = _find_terms(np.asarray(grid_u, np.float32), np.asarray(grid_w, np.float32))

    c_mask = 0
    c_post = nk
    c_ut = c_post + M
    c_frq = c_ut + G
    c_w = c_frq + J
    W = c_w + G

    in_maps = []
    for c in range(b):
        ix = idxs[c]
        pad = np.zeros(M, np.int64)
        pad[: len(ix)] = ix
        mask = np.zeros(M, np.float32)
        mask[: len(ix)] = gmask[seg[ix]].astype(np.float32)

        xs = x[pad]                       # (M, d)
        ps_ = pos[pad]                    # (M, 3)
        x_prep = np.ascontiguousarray(
            xs.reshape(nk, 128, d).transpose(1, 0, 2).reshape(128, nk * d)
        )
        aux = np.zeros((128, W), np.float32)
        aux[:, c_mask:c_mask + nk] = mask.reshape(nk, 128).T
        aux[0:3, c_post:c_post + M] = ps_.T
        aux[0:3, c_ut:c_ut + G] = np.asarray(grid_u, np.float32).T
        aux[0, c_frq:c_frq + J] = np.asarray(frequencies, np.float32)
        aux[0, c_w:c_w + G] = np.asarray(grid_w, np.float32)
        in_maps.append(dict(x=x_prep, aux=aux))

    meta = dict(n=n, p=p, s=s, f=f, d=d, b=b, G=G, J=J, nk=nk, M=M, idxs=idxs,
                terms=terms)
    return in_maps, meta


def _gather(results, meta, dtype):
    n, d, nk = meta["n"], meta["d"], meta["nk"]
    out = np.zeros((n, d), np.float32)
    for c, ix in enumerate(meta["idxs"]):
        o = results[c]["out"]                                  # (128, nk*d)
        o_nodes = o.reshape(128, nk, d).transpose(1, 0, 2).reshape(meta["M"], d)
        out[ix] = o_nodes[: len(ix)]
    return out.reshape(n, meta["p"], meta["s"], meta["f"]).astype(dtype)


def _run(inputs, positions, batch_segments, graph_mask, frequencies, grid_u,
         grid_w, trace=False, mm_dt=F32R):
    in_maps, meta = _prepare(inputs, positions, batch_segments, graph_mask,
                             frequencies, grid_u, grid_w)
    nc = _get_program(meta["nk"], meta["terms"], meta["G"], meta["J"], mm_dt)
    res = run_bass_kernel_spmd(
        nc, in_maps, core_ids=list(range(N_CORES)), trace=trace
    )
    out = _gather(res.results, meta, np.asarray(inputs).dtype)
    return out, res


def kernel(inputs, positions, batch_segments, graph_mask, frequencies, grid_u,
           grid_w):
    out, _ = _run(inputs, positions, batch_segments, graph_mask, frequencies,
                  grid_u, grid_w)
    return out
